# revision 12
# baseline (speedup 1.0000x reference)
"""Batched dynamic-weight depthwise cross-correlation on 8 trn2 NeuronCores.

out[b, y, x, c] = sum_{i,j} search[b, y+i, x+j, c] * template[b, i, j, c]
search: (128, 31, 31, 256) f32, template: (128, 7, 7, 256) f32 -> (128, 25, 25, 256) f32

Sharding: pure data parallel over batch (16 per core).

Per-core kernel (per unit = one (batch, channel-group-of-128)):
  1. DMA search/template in natural [spatial, c] layout, PE-transpose to [c, spatial].
  2. 49 taps split between:
     - DVE: fused scalar_tensor_tensor MAC (acc += s_window * t[c]), per-partition scalar.
     - PE:  diag(t[:, ij]) @ s_window matmuls in float32r accumulating in PSUM
            (diag built by ACT: eye * t[c] per-partition scale).
  3. PE transpose back to [spatial, c] with PSUM accumulation folding the
     DVE and PE partial sums, DMA out.
"""

import numpy as np

import concourse.bacc as bacc
import concourse.bass as bass
import concourse.tile as tile
from concourse import mybir
from concourse.bass_utils import run_bass_kernel_spmd

K = 7
X = 31
O = 25  # X - K + 1
B = 128
C = 256
N_CORES = 8
BL = B // N_CORES  # 16 batches per core
CG = C // 128  # 2 channel groups
F32 = mybir.dt.float32
F32R = mybir.dt.float32r

# Tap split: which of the 49 taps go to PE (diag matmul) vs DVE (STT MAC).
# Tunable; PE taps get their diag built on ACT (or DVE for a few, see DIAG_ON_DVE).
N_PE_TAPS = 35
DIAG_ON_DVE = 15  # how many of the PE-tap diags DVE builds instead of ACT

# y-split of the 625-wide output for PE matmuls (PSUM bank is 512 f32; also
# float32r needs moving dim >= 256 for full rate and EVEN column count, so
# each y-row streams 26 columns (25 useful + 1 pad read).
YSPLIT = 13  # 13*26=338 and 12*26=312 psum columns
XPAD = 26


def _build_bass():
    nc = bacc.Bacc("TRN2", target_bir_lowering=False, debug=False)

    search = nc.dram_tensor("search", [BL, X, X, C], F32, kind="ExternalInput")
    template = nc.dram_tensor("template", [BL, K, K, C], F32, kind="ExternalInput")
    eye = nc.dram_tensor("eye", [128, 128], F32, kind="ExternalInput")
    out = nc.dram_tensor("out", [BL, O, O, C], F32, kind="ExternalOutput")

    s_flat = search.ap().rearrange("b y x c -> b (y x) c")  # [BL, 961, C]
    t_flat = template.ap().rearrange("b i j c -> b (i j) c")  # [BL, 49, C]
    o_flat = out.ap().rearrange("b y x c -> b (y x) c")  # [BL, 625, C]

    taps = [(i, j) for i in range(K) for j in range(K)]
    # Interleave PE/DVE taps so both engines start early.
    pe_taps = taps[:N_PE_TAPS]
    dve_taps = taps[N_PE_TAPS:]

    SP = X * X  # 961
    in_chunks = [(k * 128, min(128, SP - k * 128)) for k in range((SP + 127) // 128)]
    OP = O * O  # 625
    out_chunks = [(k * 128, min(128, OP - k * 128)) for k in range((OP + 127) // 128)]

    with tile.TileContext(nc) as tc:
        with (
            tc.tile_pool(name="singles", bufs=1) as singles,
            tc.tile_pool(name="p_snat", bufs=6) as p_snat,
            tc.tile_pool(name="p_st", bufs=2) as p_st,
            tc.tile_pool(name="p_tnat", bufs=2) as p_tnat,
            tc.tile_pool(name="p_tt", bufs=2) as p_tt,
            tc.tile_pool(name="p_diag", bufs=4) as p_diag,
            tc.tile_pool(name="p_acc", bufs=2) as p_acc,
            tc.tile_pool(name="p_acc2", bufs=2) as p_acc2,
            tc.tile_pool(name="p_onat", bufs=6) as p_onat,
            tc.tile_pool(name="ps_tin", bufs=1, space="PSUM") as ps_tin,
            tc.tile_pool(name="ps_acc", bufs=2, space="PSUM") as ps_acc,
            tc.tile_pool(name="ps_out", bufs=2, space="PSUM") as ps_out,
        ):
            eye_sb = singles.tile([128, 128], F32)
            nc.sync.dma_start(out=eye_sb[:], in_=eye.ap()[:, :])

            for b in range(BL):
                for cg in range(CG):
                    c0 = cg * 128
                    # ---- load + transpose template slab: [49, 128] -> [128, 49]
                    t_nat = p_tnat.tile([49, 128], F32)
                    nc.sync.dma_start(
                        out=t_nat[:], in_=t_flat[b, :, c0 : c0 + 128]
                    )
                    pt_t = ps_tin.tile([128, 1024], F32, tag="ps_tin")
                    nc.tensor.transpose(
                        pt_t[:, 961 : 961 + 49], t_nat[:], eye_sb[:49, :49]
                    )
                    t_t = p_tt.tile([128, 49], F32)
                    nc.scalar.copy(out=t_t[:], in_=pt_t[:, 961 : 961 + 49])

                    # ---- load + transpose search: 8 x [rows,128] -> [128, 961]
                    # s_t is float32r-typed (the ACT evacuation rounds it so
                    # the PE-tap matmuls can consume it at full fp32r rate),
                    # laid out [c, y, 32] with a zeroed pad column so padded
                    # 26-wide window reads stay in bounds.
                    s_t = p_st.tile([128, X, 32], F32R)
                    s3r = s_t[:]
                    s3 = s_t[:].bitcast(F32)
                    for k, (r0, rows) in enumerate(in_chunks):
                        s_nat = p_snat.tile([128, 128], F32, tag="s_nat")
                        nc.sync.dma_start(
                            out=s_nat[:rows, :],
                            in_=s_flat[b, r0 : r0 + rows, c0 : c0 + 128],
                        )
                        nc.tensor.transpose(
                            pt_t[:, r0 : r0 + rows],
                            s_nat[:rows, :],
                            eye_sb[:rows, :rows],
                        )
                    pt3 = pt_t[:, :SP].rearrange("p (y x) -> p y x", x=X)
                    nc.scalar.copy(out=s3r[:, 0:16, :X], in_=pt3[:, 0:16, :])
                    nc.scalar.copy(out=s3r[:, 16:X, :X], in_=pt3[:, 16:X, :])
                    # pad column x=31: finite filler so padded window reads
                    # never hit uninitialized SBUF (values multiply into the
                    # discarded psum pad column).
                    nc.scalar.copy(
                        out=s3r[:, :, X:32],
                        in_=pt_t[:, 0:X].rearrange("p (y x) -> p y x", x=1),
                    )

                    # ---- PE taps: diag(t) @ shifted search, accumulate in PSUM
                    pa = ps_acc.tile([128, YSPLIT * XPAD], F32, tag="pa")
                    pb = ps_acc.tile([128, (O - YSPLIT) * XPAD], F32, tag="pb")
                    for n, (i, j) in enumerate(pe_taps):
                        ij = i * K + j
                        diag = p_diag.tile([128, 128], F32R, tag="diag")
                        if n < DIAG_ON_DVE:
                            nc.vector.tensor_scalar_mul(
                                out=diag[:], in0=eye_sb[:], scalar1=t_t[:, ij : ij + 1]
                            )
                        else:
                            nc.scalar.mul(
                                out=diag[:], in_=eye_sb[:], mul=t_t[:, ij : ij + 1]
                            )
                        first = n == 0
                        last = n == len(pe_taps) - 1
                        nc.tensor.matmul(
                            pa[:],
                            diag[:],
                            s3r[:, i : i + YSPLIT, j : j + XPAD],
                            start=first,
                            stop=last,
                        )
                        nc.tensor.matmul(
                            pb[:],
                            diag[:],
                            s3r[:, i + YSPLIT : i + O, j : j + XPAD],
                            start=first,
                            stop=last,
                        )
                    if pe_taps:
                        acc2 = p_acc2.tile([128, OP], F32)
                        nc.scalar.copy(
                            out=acc2[:, : YSPLIT * O].rearrange(
                                "p (y x) -> p y x", x=O
                            ),
                            in_=pa[:].rearrange("p (y x) -> p y x", x=XPAD)[:, :, :O],
                        )
                        nc.scalar.copy(
                            out=acc2[:, YSPLIT * O :].rearrange(
                                "p (y x) -> p y x", x=O
                            ),
                            in_=pb[:].rearrange("p (y x) -> p y x", x=XPAD)[:, :, :O],
                        )

                    # ---- DVE taps: fused MAC chain
                    acc = p_acc.tile([128, OP], F32)
                    a3 = acc[:].rearrange("p (y x) -> p y x", x=X - K + 1)
                    for n, (i, j) in enumerate(dve_taps):
                        ij = i * K + j
                        win = s3[:, i : i + O, j : j + O]
                        if n == 0:
                            nc.vector.tensor_scalar_mul(
                                out=a3[:], in0=win, scalar1=t_t[:, ij : ij + 1]
                            )
                        else:
                            nc.vector.scalar_tensor_tensor(
                                out=a3[:],
                                in0=win,
                                scalar=t_t[:, ij : ij + 1],
                                in1=a3[:],
                                op0=mybir.AluOpType.mult,
                                op1=mybir.AluOpType.add,
                            )

                    # ---- transpose back [128, 625] -> [625, 128] (+ sum acc2)
                    for r0, rows in out_chunks:
                        ot = ps_out.tile([128, 128], F32, tag="ot")
                        nc.tensor.matmul(
                            ot[:rows, :],
                            acc[:, r0 : r0 + rows],
                            eye_sb[:, :],
                            is_transpose=True,
                            start=True,
                            stop=not pe_taps,
                        )
                        if pe_taps:
                            nc.tensor.matmul(
                                ot[:rows, :],
                                acc2[:, r0 : r0 + rows],
                                eye_sb[:, :],
                                is_transpose=True,
                                start=False,
                                stop=True,
                            )
                        o_nat = p_onat.tile([128, 128], F32, tag="o_nat")
                        nc.scalar.copy(out=o_nat[:rows, :], in_=ot[:rows, :])
                        nc.sync.dma_start(
                            out=o_flat[b, r0 : r0 + rows, c0 : c0 + 128],
                            in_=o_nat[:rows, :],
                        )
    nc.compile()
    return nc


_NC_CACHE = None


def _get_nc():
    global _NC_CACHE
    if _NC_CACHE is None:
        _NC_CACHE = _build_bass()
    return _NC_CACHE


def _run(search: np.ndarray, template: np.ndarray, **spmd_kwargs):
    nc = _get_nc()
    search = np.ascontiguousarray(np.asarray(search), dtype=np.float32)
    template = np.ascontiguousarray(np.asarray(template), dtype=np.float32)
    eye = np.eye(128, dtype=np.float32)
    in_maps = [
        {
            "search": search[c * BL : (c + 1) * BL],
            "template": template[c * BL : (c + 1) * BL],
            "eye": eye,
        }
        for c in range(N_CORES)
    ]
    res = run_bass_kernel_spmd(nc, in_maps, core_ids=list(range(N_CORES)), **spmd_kwargs)
    out = np.concatenate([r["out"] for r in res.results], axis=0)
    return out, res


def kernel(search: np.ndarray, template: np.ndarray) -> np.ndarray:
    out, _ = _run(search, template)
    return out


# revision 17
# speedup vs baseline: 1.0283x; 1.0283x over previous
"""Batched dynamic-weight depthwise cross-correlation on 8 trn2 NeuronCores.

out[b, y, x, c] = sum_{i,j} search[b, y+i, x+j, c] * template[b, i, j, c]
search: (128, 31, 31, 256) f32, template: (128, 7, 7, 256) f32 -> (128, 25, 25, 256) f32

Sharding: pure data parallel over batch (16 per core).

Per-core kernel (per unit = one (batch, channel-group-of-128)):
  1. DMA search/template in natural [spatial, c] layout, PE-transpose to [c, spatial].
  2. 49 taps split between:
     - DVE: fused scalar_tensor_tensor MAC (acc += s_window * t[c]), per-partition scalar.
     - PE:  diag(t[:, ij]) @ s_window matmuls in float32r accumulating in PSUM
            (diag built by ACT: eye * t[c] per-partition scale).
  3. PE transpose back to [spatial, c] with PSUM accumulation folding the
     DVE and PE partial sums, DMA out.
"""

import numpy as np

import concourse.bacc as bacc
import concourse.bass as bass
import concourse.tile as tile
from concourse import mybir
from concourse.bass_utils import run_bass_kernel_spmd

K = 7
X = 31
O = 25  # X - K + 1
B = 128
C = 256
N_CORES = 8
BL = B // N_CORES  # 16 batches per core
CG = C // 128  # 2 channel groups
F32 = mybir.dt.float32
F32R = mybir.dt.float32r

# Tap split: which of the 49 taps go to PE (diag matmul) vs DVE (STT MAC).
# Tunable; PE taps get their diag built on ACT (or DVE for a few, see DIAG_ON_DVE).
N_PE_TAPS = 35
DIAG_ON_DVE = 15  # how many of the PE-tap diags DVE builds instead of ACT

# y-split of the 625-wide output for PE matmuls (PSUM bank is 512 f32; also
# float32r needs moving dim >= 256 for full rate and EVEN column count, so
# each y-row streams 26 columns (25 useful + 1 pad read).
YSPLIT = 13  # 13*26=338 and 12*26=312 psum columns
XPAD = 26

# Pool buffer counts (PSUM budget: 8 banks total).
PST_BUFS = 2
PS_TIN_BUFS = 2  # [128,1024] tile = 2 banks each
PS_ACC_BUFS = 1  # pa+pb = 2 banks each
PS_OUT_BUFS = 2  # [128,128] tile = 1 bank each
SNAT_BUFS = 6
DIAG_BUFS = 4
ONAT_BUFS = 6


def _build_bass():
    nc = bacc.Bacc("TRN2", target_bir_lowering=False, debug=False)

    search = nc.dram_tensor("search", [BL, X, X, C], F32, kind="ExternalInput")
    template = nc.dram_tensor("template", [BL, K, K, C], F32, kind="ExternalInput")
    eye = nc.dram_tensor("eye", [128, 128], F32, kind="ExternalInput")
    out = nc.dram_tensor("out", [BL, O, O, C], F32, kind="ExternalOutput")

    s_flat = search.ap().rearrange("b y x c -> b (y x) c")  # [BL, 961, C]
    t_flat = template.ap().rearrange("b i j c -> b (i j) c")  # [BL, 49, C]
    o_flat = out.ap().rearrange("b y x c -> b (y x) c")  # [BL, 625, C]

    taps = [(i, j) for i in range(K) for j in range(K)]
    # Interleave PE/DVE taps so both engines start early.
    pe_taps = taps[:N_PE_TAPS]
    dve_taps = taps[N_PE_TAPS:]

    SP = X * X  # 961
    in_chunks = [(k * 128, min(128, SP - k * 128)) for k in range((SP + 127) // 128)]
    OP = O * O  # 625
    out_chunks = [(k * 128, min(128, OP - k * 128)) for k in range((OP + 127) // 128)]

    with tile.TileContext(nc) as tc:
        with (
            tc.tile_pool(name="singles", bufs=1) as singles,
            tc.tile_pool(name="p_snat", bufs=SNAT_BUFS) as p_snat,
            tc.tile_pool(name="p_st", bufs=PST_BUFS) as p_st,
            tc.tile_pool(name="p_tnat", bufs=2) as p_tnat,
            tc.tile_pool(name="p_tt", bufs=2) as p_tt,
            tc.tile_pool(name="p_diag", bufs=DIAG_BUFS) as p_diag,
            tc.tile_pool(name="p_acc", bufs=2) as p_acc,
            tc.tile_pool(name="p_acc2", bufs=2) as p_acc2,
            tc.tile_pool(name="p_onat", bufs=ONAT_BUFS) as p_onat,
            tc.tile_pool(name="ps_tin", bufs=PS_TIN_BUFS, space="PSUM") as ps_tin,
            tc.tile_pool(name="ps_acc", bufs=PS_ACC_BUFS, space="PSUM") as ps_acc,
            tc.tile_pool(name="ps_out", bufs=PS_OUT_BUFS, space="PSUM") as ps_out,
        ):
            eye_sb = singles.tile([128, 128], F32)
            nc.sync.dma_start(out=eye_sb[:], in_=eye.ap()[:, :])

            for b in range(BL):
                for cg in range(CG):
                    c0 = cg * 128
                    # ---- load + transpose template slab: [49, 128] -> [128, 49]
                    t_nat = p_tnat.tile([49, 128], F32)
                    nc.sync.dma_start(
                        out=t_nat[:], in_=t_flat[b, :, c0 : c0 + 128]
                    )
                    pt_t = ps_tin.tile([128, 1024], F32, tag="ps_tin")
                    nc.tensor.transpose(
                        pt_t[:, 961 : 961 + 49], t_nat[:], eye_sb[:49, :49]
                    )
                    t_t = p_tt.tile([128, 49], F32)
                    nc.scalar.copy(out=t_t[:], in_=pt_t[:, 961 : 961 + 49])

                    # ---- load + transpose search: 8 x [rows,128] -> [128, 961]
                    # s_t is float32r-typed (the ACT evacuation rounds it so
                    # the PE-tap matmuls can consume it at full fp32r rate),
                    # laid out [c, y, 32] with a zeroed pad column so padded
                    # 26-wide window reads stay in bounds.
                    s_t = p_st.tile([128, X, 32], F32R)
                    s3r = s_t[:]
                    s3 = s_t[:].bitcast(F32)
                    for k, (r0, rows) in enumerate(in_chunks):
                        s_nat = p_snat.tile([128, 128], F32, tag="s_nat")
                        nc.sync.dma_start(
                            out=s_nat[:rows, :],
                            in_=s_flat[b, r0 : r0 + rows, c0 : c0 + 128],
                        )
                        nc.tensor.transpose(
                            pt_t[:, r0 : r0 + rows],
                            s_nat[:rows, :],
                            eye_sb[:rows, :rows],
                        )
                    pt3 = pt_t[:, :SP].rearrange("p (y x) -> p y x", x=X)
                    nc.scalar.copy(out=s3r[:, 0:16, :X], in_=pt3[:, 0:16, :])
                    nc.scalar.copy(out=s3r[:, 16:X, :X], in_=pt3[:, 16:X, :])
                    # pad column x=31: finite filler so padded window reads
                    # never hit uninitialized SBUF (values multiply into the
                    # discarded psum pad column).
                    nc.scalar.copy(
                        out=s3r[:, :, X:32],
                        in_=pt_t[:, 0:X].rearrange("p (y x) -> p y x", x=1),
                    )

                    # ---- PE taps: diag(t) @ shifted search, accumulate in PSUM
                    pa = ps_acc.tile([128, YSPLIT * XPAD], F32, tag="pa")
                    pb = ps_acc.tile([128, (O - YSPLIT) * XPAD], F32, tag="pb")
                    for n, (i, j) in enumerate(pe_taps):
                        ij = i * K + j
                        diag = p_diag.tile([128, 128], F32R, tag="diag")
                        if n < DIAG_ON_DVE:
                            nc.vector.tensor_scalar_mul(
                                out=diag[:], in0=eye_sb[:], scalar1=t_t[:, ij : ij + 1]
                            )
                        else:
                            nc.scalar.mul(
                                out=diag[:], in_=eye_sb[:], mul=t_t[:, ij : ij + 1]
                            )
                        first = n == 0
                        last = n == len(pe_taps) - 1
                        nc.tensor.matmul(
                            pa[:],
                            diag[:],
                            s3r[:, i : i + YSPLIT, j : j + XPAD],
                            start=first,
                            stop=last,
                        )
                        nc.tensor.matmul(
                            pb[:],
                            diag[:],
                            s3r[:, i + YSPLIT : i + O, j : j + XPAD],
                            start=first,
                            stop=last,
                        )
                    if pe_taps:
                        acc2 = p_acc2.tile([128, OP], F32)
                        nc.scalar.copy(
                            out=acc2[:, : YSPLIT * O].rearrange(
                                "p (y x) -> p y x", x=O
                            ),
                            in_=pa[:].rearrange("p (y x) -> p y x", x=XPAD)[:, :, :O],
                        )
                        nc.scalar.copy(
                            out=acc2[:, YSPLIT * O :].rearrange(
                                "p (y x) -> p y x", x=O
                            ),
                            in_=pb[:].rearrange("p (y x) -> p y x", x=XPAD)[:, :, :O],
                        )

                    # ---- DVE taps: fused MAC chain
                    acc = p_acc.tile([128, OP], F32)
                    a3 = acc[:].rearrange("p (y x) -> p y x", x=X - K + 1)
                    for n, (i, j) in enumerate(dve_taps):
                        ij = i * K + j
                        win = s3[:, i : i + O, j : j + O]
                        if n == 0:
                            nc.vector.tensor_scalar_mul(
                                out=a3[:], in0=win, scalar1=t_t[:, ij : ij + 1]
                            )
                        else:
                            nc.vector.scalar_tensor_tensor(
                                out=a3[:],
                                in0=win,
                                scalar=t_t[:, ij : ij + 1],
                                in1=a3[:],
                                op0=mybir.AluOpType.mult,
                                op1=mybir.AluOpType.add,
                            )

                    # ---- transpose back [128, 625] -> [625, 128] (+ sum acc2)
                    for r0, rows in out_chunks:
                        ot = ps_out.tile([128, 128], F32, tag="ot")
                        nc.tensor.matmul(
                            ot[:rows, :],
                            acc[:, r0 : r0 + rows],
                            eye_sb[:, :],
                            is_transpose=True,
                            start=True,
                            stop=not pe_taps,
                        )
                        if pe_taps:
                            nc.tensor.matmul(
                                ot[:rows, :],
                                acc2[:, r0 : r0 + rows],
                                eye_sb[:, :],
                                is_transpose=True,
                                start=False,
                                stop=True,
                            )
                        o_nat = p_onat.tile([128, 128], F32, tag="o_nat")
                        nc.scalar.copy(out=o_nat[:rows, :], in_=ot[:rows, :])
                        nc.sync.dma_start(
                            out=o_flat[b, r0 : r0 + rows, c0 : c0 + 128],
                            in_=o_nat[:rows, :],
                        )
    nc.compile()
    return nc


_NC_CACHE = None


def _get_nc():
    global _NC_CACHE
    if _NC_CACHE is None:
        _NC_CACHE = _build_bass()
    return _NC_CACHE


def _run(search: np.ndarray, template: np.ndarray, **spmd_kwargs):
    nc = _get_nc()
    search = np.ascontiguousarray(np.asarray(search), dtype=np.float32)
    template = np.ascontiguousarray(np.asarray(template), dtype=np.float32)
    eye = np.eye(128, dtype=np.float32)
    in_maps = [
        {
            "search": search[c * BL : (c + 1) * BL],
            "template": template[c * BL : (c + 1) * BL],
            "eye": eye,
        }
        for c in range(N_CORES)
    ]
    res = run_bass_kernel_spmd(nc, in_maps, core_ids=list(range(N_CORES)), **spmd_kwargs)
    out = np.concatenate([r["out"] for r in res.results], axis=0)
    return out, res


def kernel(search: np.ndarray, template: np.ndarray) -> np.ndarray:
    out, _ = _run(search, template)
    return out
